# revision 2
# baseline (speedup 1.0000x reference)
"""Trainium2 Bass kernel for the 2D acoustic stress-velocity FD propagator.

8 NeuronCores = 2 shots x 4 x-strips. Per core the (512,512) grid strip is
stored as (128 partitions = z-within-block, 4 z-blocks x W cols). x-derivs
are scaled-identity matmuls on column-shifted APs; z-derivs are banded
matmuls plus cross-block corrections (TensorE, float32r). Per-point
inv_rho/kappa/damp multiplies run on VectorE. Ghost columns are exchanged
every K steps core-to-core via relative remote_dma_broadcast (XOR dests,
Gray-code strip ordering). Receiver rows are DMA'd out each step.
"""

import os
import sys

if "/opt/trn_rl_repo" not in sys.path:
    sys.path.insert(0, "/opt/trn_rl_repo")

import numpy as np

# problem constants
NX = 512; NZ = 512; NT = 1000; DT = 1e-3; DX = 10.0; DZ = 10.0
C1 = 9.0 / 8.0; C2 = -1.0 / 24.0
S = 2

# layout parameters
NCX = 4
OWN = NX // NCX            # 128
K = 4                      # steps between exchanges
G = 4 * K                  # ghost width 16
PAD = 2
W = PAD + G + OWN + G + PAD   # 164
NB = 4
BP = 128
FW = NB * W                # 656
UNROLL = 2 * K             # 8
HALF = (NB // 2) * W       # 328
PSB = 256                  # psum col stride per block
EW = 3 * NB * G            # edge cols per side: 192

GRAY = [0, 1, 3, 2]        # strip pos -> core offset in group
POS = [0, 1, 3, 2]         # core offset -> strip pos (self-inverse)
LDELTA = [2, 1, 2, 1]      # by strip pos
RDELTA = [1, 2, 1, 2]

EXCHANGE = os.environ.get("AC_EXCHANGE", "rdma")
RDMA_MODE = os.environ.get("AC_RDMA_MODE", "full")  # "full" | "hs_only"
TSCALE = int(os.environ.get("AC_TSCALE", "1"))
NOREC = os.environ.get("AC_NOREC", "0") == "1"
MM_DT = os.environ.get("AC_MM_DTYPE", "f32r")

_prog_cache = {}


def _dz_mats():
    n = BP
    Df = np.zeros((n, n), np.float64)
    Db = np.zeros((n, n), np.float64)
    for i in range(n):
        for (j, c) in [(i + 1, C1), (i, -C1), (i + 2, C2), (i - 1, -C2)]:
            if 0 <= j < n:
                Df[i, j] += c
        for (j, c) in [(i, C1), (i - 1, -C1), (i + 1, C2), (i - 2, -C2)]:
            if 0 <= j < n:
                Db[i, j] += c
    return Df, Db


def _host_mats():
    """(128, 11*128): stationary operands; lhsT = M.T for out = M @ rhs."""
    sc = DX / DZ
    Df, Db = _dz_mats()
    CfU = np.zeros((BP, BP), np.float64); CfU[0, 126] = C2; CfU[0, 127] = C1; CfU[1, 127] = C2
    CfD = np.zeros((BP, BP), np.float64); CfD[127, 0] = -C2
    CbU = np.zeros((BP, BP), np.float64); CbU[0, 127] = C2 * sc
    CbD = np.zeros((BP, BP), np.float64)
    CbD[127, 0] = -C1 * sc; CbD[126, 0] = -C2 * sc; CbD[127, 1] = -C2 * sc
    eye = np.eye(BP)
    blocks = [Df.T, (Db * sc).T, CfU, CfD, CbU, CbD,
              C1 * eye, -C1 * eye, C2 * eye, -C2 * eye, eye]
    return np.ascontiguousarray(np.concatenate(blocks, axis=1)).astype(np.float32)


(M_DF, M_DBS, M_CFU, M_CFD, M_CBU, M_CBD,
 M_IC1, M_ImC1, M_IC2, M_ImC2, M_I) = range(11)


def build_program(rows):
    """rows: tuple of (block, partition) receiver rows (static)."""
    import concourse.bass as bass
    from concourse.bass import ds
    import concourse.tile as tile
    from concourse import bacc, mybir
    from contextlib import ExitStack

    f32 = mybir.dt.float32
    f32r = mybir.dt.float32r if MM_DT == "f32r" else mybir.dt.float32
    Alu = mybir.AluOpType
    nrows = len(rows)

    nc = bacc.Bacc("TRN2", target_bir_lowering=False, debug=False, num_devices=8)

    d_cA = nc.dram_tensor("cA", [BP, FW], f32, kind="ExternalInput").ap()
    d_cAZ = nc.dram_tensor("cAZ", [BP, FW], f32, kind="ExternalInput").ap()
    d_cK = nc.dram_tensor("cK", [BP, FW], f32, kind="ExternalInput").ap()
    d_cDMP = nc.dram_tensor("cDMP", [BP, FW], f32, kind="ExternalInput").ap()
    d_mats = nc.dram_tensor("mats", [BP, 11 * BP], f32, kind="ExternalInput").ap()
    d_wav = nc.dram_tensor("wav", [1, NT * TSCALE], f32, kind="ExternalInput").ap()
    d_oneP = nc.dram_tensor("oneP", [1, BP], f32, kind="ExternalInput").ap()
    d_ocol = nc.dram_tensor("ocol", [1, FW], f32, kind="ExternalInput").ap()
    d_msk = nc.dram_tensor("msk", [BP, 4], f32, kind="ExternalInput").ap()
    d_rec = nc.dram_tensor("rec", [NT * TSCALE, nrows * OWN], f32,
                           kind="ExternalOutput").ap()

    with ExitStack() as stack:
        tc = stack.enter_context(tile.TileContext(nc))
        sb = stack.enter_context(tc.tile_pool(name="sb", bufs=1))
        ps = stack.enter_context(tc.tile_pool(name="ps", bufs=1, space="PSUM"))
        sem_prep = stack.enter_context(nc.semaphore())
        sem_loc = stack.enter_context(nc.semaphore())
        sem_rem = stack.enter_context(nc.semaphore())

        FLD = sb.tile([BP, 3 * FW + 4], f32, tag="FLD")
        P_OFF, VX_OFF, VZ_OFF = 2, 2 + FW, 2 + 2 * FW

        cA = sb.tile([BP, FW], f32, tag="cA")
        cAZ = sb.tile([BP, FW], f32, tag="cAZ")
        cK = sb.tile([BP, FW], f32, tag="cK")
        cDMP = sb.tile([BP, FW], f32, tag="cDMP")
        mats = sb.tile([BP, 11 * BP], f32, tag="mats")
        matsF = sb.tile([BP, 11 * BP], f32, tag="matsF")
        ocolF = sb.tile([1, FW], f32, tag="ocolF")
        wav = sb.tile([1, NT * TSCALE], f32, tag="wav")
        oneP = sb.tile([1, BP], f32, tag="oneP")
        ocol = sb.tile([1, FW], f32, tag="ocol")
        msk = sb.tile([BP, 4], f32, tag="msk")
        wrow = sb.tile([1, BP], f32, tag="wrow")
        mA = sb.tile([BP, FW], f32, tag="mA")
        mZ = sb.tile([BP, FW], f32, tag="mZ")
        mK = sb.tile([BP, FW], f32, tag="mK")
        send = sb.tile([BP, 2 * EW], f32, tag="send")     # [L edge | R edge]
        st1_0 = sb.tile([BP, 2 * EW], f32, tag="st1_0")
        st1_1 = sb.tile([BP, 2 * EW], f32, tag="st1_1")
        st2_0 = sb.tile([BP, 2 * EW], f32, tag="st2_0")
        st2_1 = sb.tile([BP, 2 * EW], f32, tag="st2_1")
        st1 = [st1_0, st1_1]
        st2 = [st2_0, st2_1]
        tsel = sb.tile([BP, EW], f32, tag="tsel")

        nc.sync.dma_start(cA[:], d_cA)
        nc.sync.dma_start(cAZ[:], d_cAZ)
        nc.sync.dma_start(cK[:], d_cK)
        nc.sync.dma_start(cDMP[:], d_cDMP)
        nc.sync.dma_start(matsF[:], d_mats)
        nc.vector.tensor_copy(mats[:].bitcast(f32r), matsF[:])
        nc.sync.dma_start(wav[:], d_wav)
        nc.sync.dma_start(oneP[:], d_oneP)
        nc.sync.dma_start(ocolF[:], d_ocol)
        nc.vector.tensor_copy(ocol[:].bitcast(f32r), ocolF[:])
        nc.sync.dma_start(msk[:], d_msk)
        nc.vector.memset(FLD[:].bitcast(mybir.dt.uint32), 0)
        nc.vector.memset(wrow[:].bitcast(mybir.dt.uint32), 0)
        nc.gpsimd.memset(send[:], 0.0)
        for j in range(2):
            nc.gpsimd.memset(st1[j][:], 0.0)
            nc.gpsimd.memset(st2[j][:], 0.0)

        def mat(i):
            return mats[:, i * BP:(i + 1) * BP].bitcast(f32r)

        # PSUM tiles: (128, 1024) = 2 banks, block b at col PSB*b
        psA = ps.tile([BP, 1024], f32, tag="psA")
        psB = ps.tile([BP, 1024], f32, tag="psB")
        psC = ps.tile([BP, 1024], f32, tag="psC")
        psU = ps.tile([BP, 1024], f32, tag="psU")

        def ps_pair(pt, q):
            """out AP blocks {2q,2q+1}: cols 512q + {0,PSB} + [0,W)"""
            return pt.rearrange("p (x b c) -> p x b c", x=2, b=2, c=PSB)[
                :, q, :, :W]

        def ps_blocks(pt, b0, nb):
            return pt.rearrange("p (b c) -> p b c", b=4, c=PSB)[:, b0:b0 + nb, :W]

        def ps_allw(pt):
            return pt.rearrange("p (b c) -> p b c", b=4, c=PSB)[:, :, :W]

        def f_pair(off, q, shift=0):
            c0 = off + HALF * q + shift
            return FLD[:, c0: c0 + HALF]

        def f_blocks(off, b0, nb):
            return FLD[:, off + b0 * W: off + (b0 + nb) * W]

        def view_bw(t):
            return t.rearrange("p (b w) -> p b w", b=4, w=W)

        def mm(pt_ap, lhsT_i, rhs, start, stop):
            nc.tensor.matmul(pt_ap, mat(lhsT_i), rhs.bitcast(f32r),
                             start=start, stop=stop, skip_group_check=True)

        def xderiv(pt, off, fwd, start, stop=False):
            taps = ([(1, M_IC1), (0, M_ImC1), (2, M_IC2), (-1, M_ImC2)] if fwd
                    else [(0, M_IC1), (-1, M_ImC1), (1, M_IC2), (-2, M_ImC2)])
            for ti, (s, mi) in enumerate(taps):
                for q in range(2):
                    mm(ps_pair(pt, q), mi, f_pair(off, q, s),
                       start=(start and ti == 0),
                       stop=(stop and ti == len(taps) - 1))

        def zderiv(pt, off, fwd, start, stop):
            main = M_DF if fwd else M_DBS
            up = M_CFU if fwd else M_CBU
            dn = M_CFD if fwd else M_CBD
            for q in range(2):
                mm(ps_pair(pt, q), main, f_pair(off, q), start=start, stop=False)
            mm(ps_blocks(pt, 0, 2), up, f_blocks(off, 1, 2), False, False)
            mm(ps_blocks(pt, 2, 1), up, f_blocks(off, 3, 1), False, False)
            mm(ps_blocks(pt, 1, 1), dn, f_blocks(off, 0, 1), False, False)
            mm(ps_blocks(pt, 2, 2), dn, f_blocks(off, 1, 2), False, stop)

        def accum_update(pt, off, m_tile, inject):
            for q in range(2):
                mm(ps_pair(pt, q), M_I, f_pair(off, q), start=True, stop=False)
            for q in range(2):
                mm(ps_pair(pt, q), M_I, m_tile[:, HALF * q: HALF * q + HALF],
                   start=False, stop=not inject)
            if inject:
                for q in range(2):
                    nc.tensor.matmul(
                        ps_pair(pt, q), wrow[0:1, :].bitcast(f32r),
                        ocol[0:1, HALF * q: HALF * q + HALF].bitcast(f32r),
                        start=False, stop=True, skip_group_check=True)

        LEFT_OWN = PAD + G
        RIGHT_EDGE = PAD + G + OWN - G
        LGHOST = PAD
        RGHOST = PAD + G + OWN

        def edge_src(off_in_block):
            """(128, 3 fields, 4 blocks, G) src in FLD per field handled below"""
            return None

        def fld_edge(foff, w0):
            """(128, [4 blocks x G]) AP of field at `foff`, cols w0..w0+G/block"""
            return view_bw(FLD[:, foff: foff + FW])[:, :, w0:w0 + G]

        def gather_edges():
            v = nc.vector
            for fi, foff in enumerate((P_OFF, VX_OFF, VZ_OFF)):
                v.tensor_copy(
                    send[:, fi * NB * G:(fi + 1) * NB * G].rearrange(
                        "p (b g) -> p b g", b=NB, g=G),
                    fld_edge(foff, LEFT_OWN))
                v.tensor_copy(
                    send[:, EW + fi * NB * G: EW + (fi + 1) * NB * G].rearrange(
                        "p (b g) -> p b g", b=NB, g=G),
                    fld_edge(foff, RIGHT_EDGE))

        def scatter_ghosts(j):
            v = nc.vector
            s1j, s2j = st1[j], st2[j]
            # left ghost <- (stage of L-delta partner)'s RIGHT half
            v.tensor_scalar(tsel[:], s1j[:, EW:2 * EW], msk[:, 0:1], None,
                            op0=Alu.mult)
            for fi, foff in enumerate((P_OFF, VX_OFF, VZ_OFF)):
                v.scalar_tensor_tensor(
                    fld_edge(foff, LGHOST).bitcast(f32r),
                    s2j[:, EW + fi * NB * G: EW + (fi + 1) * NB * G].rearrange(
                        "p (b g) -> p b g", b=NB, g=G),
                    msk[:, 1:2],
                    tsel[:, fi * NB * G:(fi + 1) * NB * G].rearrange(
                        "p (b g) -> p b g", b=NB, g=G),
                    op0=Alu.mult, op1=Alu.add)
            # right ghost <- (stage of R-delta partner)'s LEFT half
            v.tensor_scalar(tsel[:], s1j[:, 0:EW], msk[:, 2:3], None,
                            op0=Alu.mult)
            for fi, foff in enumerate((P_OFF, VX_OFF, VZ_OFF)):
                v.scalar_tensor_tensor(
                    fld_edge(foff, RGHOST).bitcast(f32r),
                    s2j[:, fi * NB * G:(fi + 1) * NB * G].rearrange(
                        "p (b g) -> p b g", b=NB, g=G),
                    msk[:, 3:4],
                    tsel[:, fi * NB * G:(fi + 1) * NB * G].rearrange(
                        "p (b g) -> p b g", b=NB, g=G),
                    op0=Alu.mult, op1=Alu.add)

        g = nc.gpsimd
        r_prep = g.alloc_register("r_prep")
        r_loc = g.alloc_register("r_loc")
        r_rem = g.alloc_register("r_rem")

        def do_bcasts(j):
            """send `send` to xor-1 partner's st1[j] and xor-2's st2[j]."""
            with tc.tile_critical():
                g.remote_dma_broadcast(st1[j][:], send[:], sem_rem, sem_loc,
                                       rdests=[(0, 1)] + [None] * 7
                                       ).then_inc(sem_prep, 1)
                g.remote_dma_broadcast(st2[j][:], send[:], sem_rem, sem_loc,
                                       rdests=[(0, 2)] + [None] * 7
                                       ).then_inc(sem_prep, 1)
                g.reg_add(r_prep, r_prep, 2)
                g.wait_ge(sem_prep, r_prep)
                g.trigger_dma(2)
                g.reg_add(r_loc, r_loc, 32)
                g.wait_ge(sem_loc, r_loc)
                g.reg_add(r_rem, r_rem, 4)
                g.wait_ge(sem_rem, r_rem)

        def step(t_sc):
            v = nc.vector
            xderiv(psA, P_OFF, fwd=True, start=True, stop=True)
            zderiv(psB, P_OFF, fwd=True, start=True, stop=True)
            v.tensor_tensor(view_bw(mA[:].bitcast(f32r)), view_bw(cA[:]),
                            ps_allw(psA), op=Alu.mult)
            v.tensor_tensor(view_bw(mZ[:].bitcast(f32r)), view_bw(cAZ[:]),
                            ps_allw(psB), op=Alu.mult)
            accum_update(psU, VX_OFF, mA, inject=False)
            v.tensor_tensor(view_bw(FLD[:, VX_OFF:VX_OFF + FW].bitcast(f32r)),
                            view_bw(cDMP[:]), ps_allw(psU), op=Alu.mult)
            accum_update(psB, VZ_OFF, mZ, inject=False)
            v.tensor_tensor(view_bw(FLD[:, VZ_OFF:VZ_OFF + FW].bitcast(f32r)),
                            view_bw(cDMP[:]), ps_allw(psB), op=Alu.mult)
            xderiv(psC, VX_OFF, fwd=False, start=True, stop=False)
            zderiv(psC, VZ_OFF, fwd=False, start=False, stop=True)
            v.tensor_tensor(view_bw(mK[:].bitcast(f32r)), view_bw(cK[:]),
                            ps_allw(psC), op=Alu.mult)
            v.tensor_scalar(wrow[:].bitcast(f32r), oneP[:], t_sc, None,
                            op0=Alu.mult)
            accum_update(psA, P_OFF, mK, inject=True)
            v.tensor_tensor(view_bw(FLD[:, P_OFF:P_OFF + FW].bitcast(f32r)),
                            view_bw(cDMP[:]), ps_allw(psA), op=Alu.mult)

        # ---- initial handshake: sync with both partners before any data ----
        if EXCHANGE == "rdma":
            with tc.tile_critical():
                g.reg_mov(r_prep, 0)
                g.reg_mov(r_loc, 0)
                g.reg_mov(r_rem, 0)
            do_bcasts(0)

        # ---- main loop ----
        EngineType = mybir.EngineType
        with tc.For_i(0, NT * TSCALE, UNROLL,
                      hint_engines=(EngineType.PE, EngineType.DVE)) as iv:
            for k in range(UNROLL):
                step(wav[0:1, ds(iv + k, 1)])
                # record receiver rows
                for ri, (bb, pp) in enumerate(rows if not NOREC else []):
                    nc.sync.dma_start(
                        d_rec[ds(iv + k, 1), ri * OWN:(ri + 1) * OWN],
                        FLD[pp:pp + 1,
                            P_OFF + bb * W + LEFT_OWN:
                            P_OFF + bb * W + LEFT_OWN + OWN])
                if EXCHANGE == "rdma" and RDMA_MODE == "full" and (k + 1) % K == 0:
                    j = (k + 1) // K - 1
                    gather_edges()
                    do_bcasts(j)
                    scatter_ghosts(j)

    nc.compile()
    return nc


def _mk_tile(a2d, q, fill=0.0):
    """(128, FW) per-core tile from (NZ, NX) array; strip pos q."""
    x0 = q * OWN - (PAD + G)
    t = np.full((BP, FW), fill, np.float32)
    cols = np.arange(W)
    gx = x0 + cols
    valid = (gx >= 0) & (gx < NX)
    gxc = np.clip(gx, 0, NX - 1)
    for bb in range(NB):
        sl = a2d[bb * BP:(bb + 1) * BP, :]
        v = np.where(valid[None, :], sl[:, gxc], fill)
        t[:, bb * W:(bb + 1) * W] = v
    return t.astype(np.float32)


def kernel(**inputs):
    from concourse.bass_utils import run_bass_kernel_spmd

    vp = np.asarray(inputs["vp"], np.float32)
    rho = np.asarray(inputs["rho"], np.float32)
    damp = np.asarray(inputs["damp"], np.float32)
    wavelet = np.asarray(inputs["wavelet"], np.float32)
    src_x = np.asarray(inputs["src_x"]); src_z = np.asarray(inputs["src_z"])
    rcv_x = np.asarray(inputs["rcv_x"]); rcv_z = np.asarray(inputs["rcv_z"])

    kappa = (rho * vp * vp).astype(np.float64)
    inv_rho = (1.0 / rho).astype(np.float64)

    row_list = sorted(set(int(z) for z in rcv_z))
    assert len(row_list) <= 4, "too many distinct receiver rows"
    rows = tuple((rz // BP, rz % BP) for rz in row_list)

    key = (rows, EXCHANGE, MM_DT, RDMA_MODE, TSCALE, NOREC)
    if key not in _prog_cache:
        _prog_cache[key] = build_program(rows)
    nc = _prog_cache[key]

    mats = _host_mats()
    in_maps = []
    for c in range(8):
        shot = c // 4
        q = POS[c % 4]
        cA = _mk_tile((DT / DX) * inv_rho, q)
        cAZ = _mk_tile((DT / DZ) * inv_rho, q)
        cK = _mk_tile((DT / DX) * kappa, q)
        cDMP = _mk_tile(damp.astype(np.float64), q)
        pad_cols = np.concatenate([np.arange(PAD), np.arange(W - PAD, W)])
        for bb in range(NB):
            cDMP[:, bb * W + pad_cols] = 0.0
        oneP = np.zeros((1, BP), np.float32)
        ocol = np.zeros((1, FW), np.float32)
        sx, sz = int(src_x[shot]), int(src_z[shot])
        lw = sx - (q * OWN - (PAD + G))
        if PAD + G <= lw < PAD + G + OWN:
            amp = DT / (DX * DZ) / float(damp[sz, sx])
            oneP[0, sz % BP] = 1.0
            ocol[0, (sz // BP) * W + lw] = amp
        mskv = np.zeros((BP, 4), np.float32)
        mskv[:, 0] = 1.0 if LDELTA[q] == 1 else 0.0
        mskv[:, 1] = 1.0 - mskv[0, 0]
        mskv[:, 2] = 1.0 if RDELTA[q] == 1 else 0.0
        mskv[:, 3] = 1.0 - mskv[0, 2]
        in_maps.append({
            "cA": cA, "cAZ": cAZ, "cK": cK, "cDMP": cDMP, "mats": mats,
            "wav": np.tile(wavelet[shot:shot + 1, :].astype(np.float32), (1, TSCALE)),
            "oneP": oneP, "ocol": ocol, "msk": mskv,
        })

    trace = os.environ.get("AC_TRACE", "0") == "1"
    res = run_bass_kernel_spmd(nc, in_maps, core_ids=list(range(8)),
                               trace=trace)
    global LAST_EXEC_NS, LAST_RESULT
    LAST_EXEC_NS = getattr(res, "exec_time_ns", None)
    LAST_RESULT = res

    out = np.zeros((S, NT, len(rcv_x)), np.float32)
    rows_full = {}
    for c in range(8):
        shot = c // 4
        q = POS[c % 4]
        rec = np.asarray(res.results[c]["rec"])[:NT]  # (NT, nrows*OWN)
        for ri, rz in enumerate(row_list):
            rows_full.setdefault((shot, rz), np.zeros((NT, NX), np.float32))[
                :, q * OWN:(q + 1) * OWN] = rec[:, ri * OWN:(ri + 1) * OWN]
    for r in range(len(rcv_x)):
        rz = int(rcv_z[r]); rx = int(rcv_x[r])
        for shot in range(S):
            out[shot, :, r] = rows_full[(shot, rz)][:, rx]
    return out


if __name__ == "__main__":
    print("kernel module ok")

